# revision 4
# baseline (speedup 1.0000x reference)
"""GQA kernel for 8 trn2 NeuronCores — v3 (DP x TP).

Sharding: 2-way data-parallel over batch x 4-way tensor-parallel over KV
groups. Core c owns batch c//4 and KV group g=c%4: kv heads {2g, 2g+1},
q heads {8g..8g+7} (cols 512g:512g+512 of Wq, cols 128g:128g+128 of
Wk/Wv, rows 512g:512g+512 of Wo). Each core computes a partial output
[S, E]; host sums 4 partials per batch. Per-core input DMA: one batch
only (25 MB vs 50 MB).

Device algorithm (single batch per core):
  A. t-quarter pipelined: per t (512 seq): DMA kT/vT/qT chunks
     [128, 512], project K/V (M=128: both kv heads packed per matmul)
     and Q pairs for t=0. K.T per kv head duplicated into the other
     64-partition half (SBUF->SBUF DMA) so score matmuls row-tile both
     heads of a pair; V -> natural + ones column -> V_aug [S, 65].
  B. jq-outer, pair-inner attention; per (pair, jq) 8 groups of 2 kv
     chunks: row-tiled score pair -> exp (ACT, PSUM->SBUF bf16) ->
     ctx.T accumulation (row 64 = softmax denominator). Between groups,
     PE-filler units are interleaved: Q projection for t=jq+1 and
     out-projection (C) chunks of jq-1 — keeping PE busy while ACT
     (exp) drains, since exp can only run on the Activation engine.
  C. out chunks: out[t*128:+128, e*512:+512] = sum_p ctxT_p.T @ Wo_p,
     emitted as fillers during B (tail for jq=3 at the end).

All matmuls bf16 inputs / fp32 PSUM. PSUM banks: acc(2x1) + sc(2x2) +
ctx(2) = 8.
"""

import numpy as np
import ml_dtypes

B = 2
S = 2048
E = 2048
HD = 64           # head dim
HPC = 8           # q heads per core
NP = 4            # head pairs per core
NKVH = 2          # kv heads per core
QD = HPC * HD     # 512 per-core q dims
KD = NKVH * HD    # 128 per-core kv dims
NCORES = 8
NTPG = 4          # tensor-parallel group size (cores per batch)
EC = E // 128     # 16 contraction chunks
NJQ = S // 512    # 4 q-chunks of 512
NKV = S // 128    # 16 kv chunks of 128
KVG = 2           # kv chunks per exp group
BF16 = ml_dtypes.bfloat16

_cache = {}


def _build():
    from contextlib import ExitStack
    from concourse import bacc, tile
    import concourse.mybir as mybir

    bf16 = mybir.dt.bfloat16
    f32 = mybir.dt.float32
    EXP = mybir.ActivationFunctionType.Exp

    nc = bacc.Bacc(
        "TRN2", target_bir_lowering=False, debug=False, num_devices=NCORES)
    qT_d = nc.declare_dram_parameter("qT", [E, S], bf16, isOutput=False)
    kT_d = nc.declare_dram_parameter("kT", [E, S], bf16, isOutput=False)
    vT_d = nc.declare_dram_parameter("vT", [E, S], bf16, isOutput=False)
    wq_d = nc.declare_dram_parameter("wq", [E, QD], bf16, isOutput=False)
    wk_d = nc.declare_dram_parameter("wk", [E, KD], bf16, isOutput=False)
    wv_d = nc.declare_dram_parameter("wv", [E, KD], bf16, isOutput=False)
    wo_d = nc.declare_dram_parameter("wo", [QD, E], bf16, isOutput=False)
    out_d = nc.declare_dram_parameter("out", [S, E], bf16, isOutput=True)

    with ExitStack() as ctx:
        tc = ctx.enter_context(tile.TileContext(nc))
        # ---- pools ----
        wpool = ctx.enter_context(tc.tile_pool(name="w", bufs=1))
        qin = ctx.enter_context(tc.tile_pool(name="qin", bufs=64))
        kvin = ctx.enter_context(tc.tile_pool(name="kvin", bufs=6))
        qts = ctx.enter_context(tc.tile_pool(name="qts", bufs=1))
        vnp = ctx.enter_context(tc.tile_pool(name="vnp", bufs=32))
        ptp = ctx.enter_context(tc.tile_pool(name="ptp", bufs=4))
        ostp = ctx.enter_context(tc.tile_pool(name="ostp", bufs=3))
        smp = ctx.enter_context(tc.tile_pool(name="smp", bufs=2))
        psa = ctx.enter_context(tc.tile_pool(name="psa", bufs=2, space="PSUM"))
        psc = ctx.enter_context(tc.tile_pool(name="psc", bufs=2, space="PSUM"))

        # ---- weights (loaded once) ----
        wq_sb = wpool.tile([128, EC, QD], bf16)
        nc.sync.dma_start(wq_sb[:], wq_d.rearrange("(c p) m -> p c m", p=128))
        wk_sb = wpool.tile([128, EC, KD], bf16)
        nc.sync.dma_start(wk_sb[:], wk_d.rearrange("(c p) m -> p c m", p=128))
        wv_sb = wpool.tile([128, EC, KD], bf16)
        nc.sync.dma_start(wv_sb[:], wv_d.rearrange("(c p) m -> p c m", p=128))
        wo_sb = wpool.tile([128, NP, E], bf16)
        nc.sync.dma_start(wo_sb[:], wo_d.rearrange("(c p) e -> p c e", p=128))

        # ---- persistent SBUF tiles ----
        # Q pair tiles: heads 2p at partitions 0:64, 2p+1 at 64:128
        qp_sb = [qts.tile([128, S], bf16, tag=f"qp{p}", name=f"qp{p}")
                 for p in range(NP)]
        # K.T per kv head, duplicated across both partition halves
        kt2_sb = [qts.tile([128, S], bf16, tag=f"kt2{h}", name=f"kt2{h}")
                  for h in range(NKVH)]
        # V.T packed: head 0 at partitions 0:64, head 1 at 64:128
        vt_sb = qts.tile([128, S], bf16, tag="vt", name="vt")
        # ctx.T pair tiles
        ctxT_sb = [qts.tile([128, S], bf16, tag=f"ctxT{p}", name=f"ctxT{p}")
                   for p in range(NP)]
        # V natural + ones col, per kv head per kv chunk
        vn_tiles = [[None] * NKV for _ in range(NKVH)]
        # qT chunk tiles [128, 512] per (e, t), kept resident for Q proj
        qt_tiles = [[None] * NJQ for _ in range(EC)]

        def dma_q_quarter(t):
            for e in range(EC):
                qt = qin.tile([128, 512], bf16, tag="qt", name="qt")
                nc.sync.dma_start(
                    qt[:], qT_d[e * 128:(e + 1) * 128, t * 512:(t + 1) * 512])
                qt_tiles[e][t] = qt

        def q_unit(p, t):
            """Project Q pair p at seq quarter t (atomic: 16 matmuls)."""
            acc = psa.tile([128, 512], f32, tag="acc", name="qacc")
            for e in range(EC):
                nc.tensor.matmul(
                    acc[:], lhsT=wq_sb[:, e, p * 128:(p + 1) * 128],
                    rhs=qt_tiles[e][t][:],
                    start=(e == 0), stop=(e == EC - 1))
            nc.vector.tensor_scalar_mul(
                qp_sb[p][:, t * 512:(t + 1) * 512], acc[:], 0.125)

        def kv_quarter(t):
            """DMA + project K and V at seq quarter t (M=128: both kv
            heads packed)."""
            kins, vins = [], []
            for e in range(EC):
                ki = kvin.tile([128, 512], bf16, tag="kin", name="kin")
                nc.sync.dma_start(
                    ki[:], kT_d[e * 128:(e + 1) * 128, t * 512:(t + 1) * 512])
                kins.append(ki)
                vi = kvin.tile([128, 512], bf16, tag="vin", name="vin")
                nc.sync.dma_start(
                    vi[:], vT_d[e * 128:(e + 1) * 128, t * 512:(t + 1) * 512])
                vins.append(vi)
            kacc = psa.tile([128, 512], f32, tag="acc", name="kacc")
            for e in range(EC):
                nc.tensor.matmul(kacc[:], lhsT=wk_sb[:, e, :], rhs=kins[e][:],
                                 start=(e == 0), stop=(e == EC - 1))
            # head 0 -> kt2[0] partitions 0:64, head 1 -> kt2[1] 64:128
            nc.vector.tensor_copy(
                kt2_sb[0][0:64, t * 512:(t + 1) * 512], kacc[0:64, :])
            nc.vector.tensor_copy(
                kt2_sb[1][64:128, t * 512:(t + 1) * 512], kacc[64:128, :])
            vacc = psa.tile([128, 512], f32, tag="acc", name="vacc")
            for e in range(EC):
                nc.tensor.matmul(vacc[:], lhsT=wv_sb[:, e, :], rhs=vins[e][:],
                                 start=(e == 0), stop=(e == EC - 1))
            nc.vector.tensor_copy(vt_sb[:, t * 512:(t + 1) * 512], vacc[:])
            # V natural + ones column per kv chunk of this quarter
            for c in range(t * 4, t * 4 + 4):
                for h in range(NKVH):
                    vn = vnp.tile([128, HD + 1], bf16, tag="vn", name="vn")
                    nc.vector.memset(vn[:, HD:HD + 1], 1.0)
                    nc.sync.dma_start_transpose(
                        out=vn[:, 0:HD],
                        in_=vt_sb[h * 64:(h + 1) * 64, c * 128:(c + 1) * 128])
                    vn_tiles[h][c] = vn

        def c_unit(t, e):
            """out[t*128:+128, e*512:+512]: 4 accum matmuls + copy + DMA."""
            ops = psa.tile([128, 512], f32, tag="acc", name="cacc")
            for kc in range(NP):
                nc.tensor.matmul(
                    ops[:], lhsT=ctxT_sb[kc][:, t * 128:(t + 1) * 128],
                    rhs=wo_sb[:, kc, e * 512:(e + 1) * 512],
                    start=(kc == 0), stop=(kc == NP - 1))
            ost = ostp.tile([128, 512], bf16, tag="ost", name="ost")
            nc.vector.tensor_copy(ost[:], ops[:])
            nc.sync.dma_start(
                out_d[t * 128:(t + 1) * 128, e * 512:(e + 1) * 512], ost[:])

        # ---------- Phase A ----------
        for t in range(NJQ):
            kv_quarter(t)
            dma_q_quarter(t)
            if t == 0:
                for p in range(NP):
                    q_unit(p, 0)
        # duplicate K.T halves so score pairs can row-tile
        nc.sync.dma_start(kt2_sb[0][64:128, :], kt2_sb[0][0:64, :])
        nc.sync.dma_start(kt2_sb[1][0:64, :], kt2_sb[1][64:128, :])

        # ---------- Phase B with interleaved PE-filler units ----------
        def b_group(p, jq, g, ctx_ps, h):
            sc_e = psc.tile([128, KVG * 512], f32, tag="sc", name="sc_e")
            sc_o = psc.tile([128, KVG * 512], f32, tag="sc", name="sc_o")
            for ki in range(KVG):
                kv = g * KVG + ki
                nc.tensor.matmul(
                    sc_e[:, ki * 512:(ki + 1) * 512],
                    lhsT=kt2_sb[h][0:64, kv * 128:(kv + 1) * 128],
                    rhs=qp_sb[p][0:64, jq * 512:(jq + 1) * 512],
                    start=True, stop=True)
                nc.tensor.matmul(
                    sc_o[:, ki * 512:(ki + 1) * 512],
                    lhsT=kt2_sb[h][64:128, kv * 128:(kv + 1) * 128],
                    rhs=qp_sb[p][64:128, jq * 512:(jq + 1) * 512],
                    start=True, stop=True)
            pt_e = ptp.tile([128, KVG * 512], bf16, tag="pt", name="pt_e")
            nc.scalar.activation(pt_e[:], sc_e[:], EXP)
            pt_o = ptp.tile([128, KVG * 512], bf16, tag="pt", name="pt_o")
            nc.scalar.activation(pt_o[:], sc_o[:], EXP)
            for ki in range(KVG):
                kv = g * KVG + ki
                nc.tensor.matmul(
                    ctx_ps[0:HD + 1, 0:512],
                    lhsT=vn_tiles[h][kv][:, 0:HD + 1],
                    rhs=pt_e[:, ki * 512:(ki + 1) * 512],
                    start=(kv == 0), stop=(kv == NKV - 1))
                nc.tensor.matmul(
                    ctx_ps[0:HD + 1, 512:1024],
                    lhsT=vn_tiles[h][kv][:, 0:HD + 1],
                    rhs=pt_o[:, ki * 512:(ki + 1) * 512],
                    start=(kv == 0), stop=(kv == NKV - 1))

        def normalize(p, jq, ctx_ps):
            for hp in range(2):
                recip = smp.tile([1, 512], f32, tag="recip", name="recip")
                nc.vector.reciprocal(
                    recip[:], ctx_ps[HD:HD + 1, hp * 512:(hp + 1) * 512])
                rb = smp.tile([64, 512], f32, tag="rb", name="rb")
                nc.gpsimd.partition_broadcast(rb[:], recip[:])
                nc.vector.tensor_mul(
                    ctxT_sb[p][hp * 64:(hp + 1) * 64,
                               jq * 512:(jq + 1) * 512],
                    ctx_ps[0:64, hp * 512:(hp + 1) * 512], rb[:])

        for jq in range(NJQ):
            units = []
            if jq + 1 < NJQ:
                units += [(q_unit, (p, jq + 1)) for p in range(NP)]
            if jq > 0:
                units += [(c_unit, (t, e))
                          for t in range((jq - 1) * 4, jq * 4)
                          for e in range(E // 512)]
            emitted = 0
            ngroups = NP * (NKV // KVG)
            gi = 0
            for p in range(NP):
                h = p // 2
                ctx_ps = psc.tile([128, 1024], f32, tag="ctx", bufs=1,
                                  name="ctx_ps")
                for g in range(NKV // KVG):
                    b_group(p, jq, g, ctx_ps, h)
                    gi += 1
                    want = (gi * len(units)) // ngroups
                    while emitted < want:
                        fn, args = units[emitted]
                        fn(*args)
                        emitted += 1
                normalize(p, jq, ctx_ps)
        for t in range((NJQ - 1) * 4, NJQ * 4):
            for e in range(E // 512):
                c_unit(t, e)
    nc.compile()
    return nc


def _get_nc():
    if "nc" not in _cache:
        _cache["nc"] = _build()
    return _cache["nc"]


def kernel(query, key, value, Wq, Wk, Wv, Wo, _trace=False):
    from concourse.bass_utils import run_bass_kernel_spmd

    def t_bf16(x):
        return np.ascontiguousarray(
            np.asarray(x, np.float32).astype(BF16).transpose(0, 2, 1))

    qT = t_bf16(query)
    kT = t_bf16(key)
    vT = t_bf16(value)
    Wq = np.asarray(Wq, np.float32).astype(BF16)
    Wk = np.asarray(Wk, np.float32).astype(BF16)
    Wv = np.asarray(Wv, np.float32).astype(BF16)
    Wo = np.asarray(Wo, np.float32).astype(BF16)

    in_maps = []
    for c in range(NCORES):
        b, g = divmod(c, NTPG)
        in_maps.append({
            "qT": np.ascontiguousarray(qT[b]),
            "kT": np.ascontiguousarray(kT[b]),
            "vT": np.ascontiguousarray(vT[b]),
            "wq": np.ascontiguousarray(Wq[:, g * QD:(g + 1) * QD]),
            "wk": np.ascontiguousarray(Wk[:, g * KD:(g + 1) * KD]),
            "wv": np.ascontiguousarray(Wv[:, g * KD:(g + 1) * KD]),
            "wo": np.ascontiguousarray(Wo[g * QD:(g + 1) * QD, :]),
        })

    nc = _get_nc()
    res = run_bass_kernel_spmd(nc, in_maps, list(range(NCORES)), trace=_trace)
    out = np.zeros((B, S, E), np.float32)
    for c in range(NCORES):
        out[c // NTPG] += res.results[c]["out"].astype(np.float32)
    if _trace:
        _cache["last_exec_time_ns"] = res.exec_time_ns
        _cache["last_results"] = res
    return out


# revision 17
# speedup vs baseline: 1.2076x; 1.2076x over previous
"""GQA kernel for 8 trn2 NeuronCores — v6 (DP x TP).

Sharding: 2-way data-parallel over batch x 4-way tensor-parallel over KV
groups. Core c owns batch c//4 and KV group g=c%4: kv heads {2g, 2g+1},
q heads {8g..8g+7} (cols 512g:512g+512 of Wq, cols 128g:128g+128 of
Wk/Wv, rows 512g:512g+512 of Wo). Each core computes a partial output
[S, E]; host sums 4 partials per batch.

DMA instruction count is minimized (HWDGE costs ~630ns serialized per
DMA): inputs land as [128, 4, 512] 4-chunk groups, weights in 1-4
transfers each, outputs batched per 128-row slab.

Device algorithm (single batch per core):
  A. per seq-quarter t: DMA kT/vT/qT groups; K.T projection (M=128,
     both kv heads packed), V projected DIRECTLY in natural layout
     (lhsT = vT chunk, rhs = Wv) -> per-chunk per-head V_aug [128, 65]
     tiles (ones col appended) with no DMA transposes; Q pairs
     projected as the needed qT groups arrive. K.T per head duplicated
     into the other partition half per quarter (SBUF->SBUF DMA) so
     score matmuls can row-tile both heads of a pair. Pair-0 jq-0
     attention groups are trickled between quarters to fill PE.
  B. remaining (pair, jq); per (pair, jq) 8 groups of 2 kv chunks:
     row-tiled score pair -> exp (ACT, PSUM->SBUF bf16) -> ctx.T
     accumulation (row 64 = softmax denominator). Between groups,
     PE-filler units (Q proj t=3, out-proj chunks of jq-1) keep PE busy
     while ACT drains exp (exp runs only on the Activation engine).
  C. out slab t: out[t*128:+128, :] = sum_p ctxT_p.T @ Wo_p, 4 PSUM
     accs + copies into one [128, 2048] tile, single DMA.

All matmuls bf16 inputs / fp32 PSUM. PSUM banks: acc(2x1) + sc(2x2) +
ctx(2) = 8.
"""

import numpy as np
import ml_dtypes

B = 2
S = 2048
E = 2048
HD = 64           # head dim
HPC = 8           # q heads per core
NP = 4            # head pairs per core
NKVH = 2          # kv heads per core
QD = HPC * HD     # 512 per-core q dims
KD = NKVH * HD    # 128 per-core kv dims
NCORES = 8
NTPG = 4          # tensor-parallel group size (cores per batch)
EC = E // 128     # 16 contraction chunks
EG = 4            # e-chunks per input DMA group
NEG = EC // EG    # 4 groups
NJQ = S // 512    # 4 q-chunks of 512
NKV = S // 128    # 16 kv chunks of 128
KVG = 2           # kv chunks per exp group
BF16 = ml_dtypes.bfloat16

_cache = {}


def _build():
    from contextlib import ExitStack
    from concourse import bacc, tile
    import concourse.mybir as mybir

    bf16 = mybir.dt.bfloat16
    f32 = mybir.dt.float32
    EXP = mybir.ActivationFunctionType.Exp

    nc = bacc.Bacc(
        "TRN2", target_bir_lowering=False, debug=False, num_devices=NCORES)
    qT_d = nc.declare_dram_parameter("qT", [E, S], bf16, isOutput=False)
    kT_d = nc.declare_dram_parameter("kT", [E, S], bf16, isOutput=False)
    vT_d = nc.declare_dram_parameter("vT", [E, S], bf16, isOutput=False)
    wq_d = nc.declare_dram_parameter("wq", [E, QD], bf16, isOutput=False)
    wk_d = nc.declare_dram_parameter("wk", [E, KD], bf16, isOutput=False)
    wv_d = nc.declare_dram_parameter("wv", [E, KD], bf16, isOutput=False)
    wo_d = nc.declare_dram_parameter("wo", [QD, E], bf16, isOutput=False)
    out_d = nc.declare_dram_parameter("out", [S, E], bf16, isOutput=True)

    with ExitStack() as ctx:
        tc = ctx.enter_context(tile.TileContext(nc))
        # ---- pools ----
        wpool = ctx.enter_context(tc.tile_pool(name="w", bufs=1))
        qin = ctx.enter_context(tc.tile_pool(name="qin", bufs=16))
        kvin = ctx.enter_context(tc.tile_pool(name="kvin", bufs=4))
        qts = ctx.enter_context(tc.tile_pool(name="qts", bufs=1))
        vnp = ctx.enter_context(tc.tile_pool(name="vnp", bufs=32))
        ptp = ctx.enter_context(tc.tile_pool(name="ptp", bufs=4))
        ostp = ctx.enter_context(tc.tile_pool(name="ostp", bufs=2))
        smp = ctx.enter_context(tc.tile_pool(name="smp", bufs=2))
        psa = ctx.enter_context(tc.tile_pool(name="psa", bufs=2, space="PSUM"))
        psc = ctx.enter_context(tc.tile_pool(name="psc", bufs=2, space="PSUM"))

        # ---- weights: few big DMAs on the ACT queue (idle in phase A) ----
        wk_sb = wpool.tile([128, EC, KD], bf16)
        nc.scalar.dma_start(wk_sb[:], wk_d.rearrange("(c p) m -> p c m", p=128))
        wv_sb = wpool.tile([128, EC, KD], bf16)
        nc.scalar.dma_start(wv_sb[:], wv_d.rearrange("(c p) m -> p c m", p=128))
        wq_sb = wpool.tile([128, EC, QD], bf16)
        wq_r = wq_d.rearrange("(c p) m -> p c m", p=128)
        wo_sb = wpool.tile([128, NP, E], bf16)
        wo_r = wo_d.rearrange("(c p) e -> p c e", p=128)

        def dma_wq():
            for g in range(NEG):
                nc.scalar.dma_start(
                    wq_sb[:, g * EG:(g + 1) * EG, :], wq_r[:, g * EG:(g + 1) * EG, :])

        def dma_wo():
            for kc in range(0, NP, 2):
                nc.scalar.dma_start(
                    wo_sb[:, kc:kc + 2, :], wo_r[:, kc:kc + 2, :])

        # ---- persistent SBUF tiles ----
        # Q pair tiles: heads 2p at partitions 0:64, 2p+1 at 64:128
        qp_sb = [qts.tile([128, S], bf16, tag=f"qp{p}", name=f"qp{p}")
                 for p in range(NP)]
        # K.T per kv head, duplicated across both partition halves
        kt2_sb = [qts.tile([128, S], bf16, tag=f"kt2{h}", name=f"kt2{h}")
                  for h in range(NKVH)]
        # ctx.T pair tiles
        ctxT_sb = [qts.tile([128, S], bf16, tag=f"ctxT{p}", name=f"ctxT{p}")
                   for p in range(NP)]
        # V natural + ones col, per kv head per kv chunk
        vn_tiles = [[None] * NKV for _ in range(NKVH)]
        # qT group tiles [128, EG, 512] per (g, t), resident for Q proj
        qt_tiles = [[None] * NJQ for _ in range(NEG)]

        def dma_q_quarter(t):
            # qT on the ACT queue: fully resident (16 bufs), no ring waits.
            for g in range(NEG):
                qt = qin.tile([128, EG, 512], bf16, tag="qt", name="qt")
                nc.scalar.dma_start(
                    qt[:],
                    qT_d[g * EG * 128:(g + 1) * EG * 128,
                         t * 512:(t + 1) * 512].rearrange(
                             "(g p) s -> p g s", p=128))
                qt_tiles[g][t] = qt

        def q_unit(p, t):
            """Project Q pair p at seq quarter t (atomic: 16 matmuls)."""
            acc = psa.tile([128, 512], f32, tag="acc", name="qacc")
            for e in range(EC):
                nc.tensor.matmul(
                    acc[:], lhsT=wq_sb[:, e, p * 128:(p + 1) * 128],
                    rhs=qt_tiles[e // EG][t][:, e % EG, :],
                    start=(e == 0), stop=(e == EC - 1))
            nc.vector.tensor_scalar_mul(
                qp_sb[p][:, t * 512:(t + 1) * 512], acc[:], 0.125)

        def kv_quarter(t):
            """DMA + project K (transposed) and V (natural) at quarter t."""
            kins, vins = [], []
            for g in range(NEG):
                ki = kvin.tile([128, EG, 512], bf16, tag="kin", name="kin")
                nc.sync.dma_start(
                    ki[:],
                    kT_d[g * EG * 128:(g + 1) * EG * 128,
                         t * 512:(t + 1) * 512].rearrange(
                             "(g p) s -> p g s", p=128))
                kins.append(ki)
                vi = kvin.tile([128, EG, 512], bf16, tag="vin", name="vin")
                nc.sync.dma_start(
                    vi[:],
                    vT_d[g * EG * 128:(g + 1) * EG * 128,
                         t * 512:(t + 1) * 512].rearrange(
                             "(g p) s -> p g s", p=128))
                vins.append(vi)
            kacc = psa.tile([128, 512], f32, tag="acc", name="kacc")
            for e in range(EC):
                nc.tensor.matmul(kacc[:], lhsT=wk_sb[:, e, :],
                                 rhs=kins[e // EG][:, e % EG, :],
                                 start=(e == 0), stop=(e == EC - 1))
            # head 0 -> kt2[0] partitions 0:64, head 1 -> kt2[1] 64:128
            nc.vector.tensor_copy(
                kt2_sb[0][0:64, t * 512:(t + 1) * 512], kacc[0:64, :])
            nc.vector.tensor_copy(
                kt2_sb[1][64:128, t * 512:(t + 1) * 512], kacc[64:128, :])
            # V natural: per kv chunk, out[seq 128, kvdim 128] then split
            # per head into V_aug tiles (ones column appended).
            for c in range(t * 4, t * 4 + 4):
                vacc = psa.tile([128, 512], f32, tag="acc", name="vacc")
                for e in range(EC):
                    nc.tensor.matmul(
                        vacc[:, 0:KD],
                        lhsT=vins[e // EG][:, e % EG,
                                           (c % 4) * 128:(c % 4) * 128 + 128],
                        rhs=wv_sb[:, e, :],
                        start=(e == 0), stop=(e == EC - 1))
                for h in range(NKVH):
                    vn = vnp.tile([128, HD + 1], bf16, tag="vn", name="vn")
                    nc.vector.tensor_copy(
                        vn[:, 0:HD], vacc[:, h * HD:(h + 1) * HD])
                    nc.vector.memset(vn[:, HD:HD + 1], 1.0)
                    vn_tiles[h][c] = vn
            # per-quarter K.T duplication so pair-0 attention can start
            # before the full K row is assembled
            nc.sync.dma_start(kt2_sb[0][64:128, t * 512:(t + 1) * 512],
                              kt2_sb[0][0:64, t * 512:(t + 1) * 512])
            nc.sync.dma_start(kt2_sb[1][0:64, t * 512:(t + 1) * 512],
                              kt2_sb[1][64:128, t * 512:(t + 1) * 512])

        def c_slab(t):
            """out[t*128:+128, :]: 4 accs + 4 copies into one tile, 1 DMA."""
            ost = ostp.tile([128, E], bf16, tag="ost", name="ost")
            for e in range(E // 512):
                ops = psa.tile([128, 512], f32, tag="acc", name="cacc")
                for kc in range(NP):
                    nc.tensor.matmul(
                        ops[:], lhsT=ctxT_sb[kc][:, t * 128:(t + 1) * 128],
                        rhs=wo_sb[:, kc, e * 512:(e + 1) * 512],
                        start=(kc == 0), stop=(kc == NP - 1))
                nc.vector.tensor_copy(ost[:, e * 512:(e + 1) * 512], ops[:])
            nc.sync.dma_start(out_d[t * 128:(t + 1) * 128, :], ost[:])

        def c_unit(t, e, ost):
            ops = psa.tile([128, 512], f32, tag="acc", name="cacc")
            for kc in range(NP):
                nc.tensor.matmul(
                    ops[:], lhsT=ctxT_sb[kc][:, t * 128:(t + 1) * 128],
                    rhs=wo_sb[:, kc, e * 512:(e + 1) * 512],
                    start=(kc == 0), stop=(kc == NP - 1))
            nc.vector.tensor_copy(ost[:, e * 512:(e + 1) * 512], ops[:])
            if e == E // 512 - 1:
                nc.sync.dma_start(out_d[t * 128:(t + 1) * 128, :], ost[:])

        # ---------- Phase B building blocks ----------
        def b_group(p, jq, g, ctx_ps, h):
            sc_e = psc.tile([128, KVG * 512], f32, tag="sc", name="sc_e")
            sc_o = psc.tile([128, KVG * 512], f32, tag="sc", name="sc_o")
            for ki in range(KVG):
                kv = g * KVG + ki
                nc.tensor.matmul(
                    sc_e[:, ki * 512:(ki + 1) * 512],
                    lhsT=kt2_sb[h][0:64, kv * 128:(kv + 1) * 128],
                    rhs=qp_sb[p][0:64, jq * 512:(jq + 1) * 512],
                    start=True, stop=True)
                nc.tensor.matmul(
                    sc_o[:, ki * 512:(ki + 1) * 512],
                    lhsT=kt2_sb[h][64:128, kv * 128:(kv + 1) * 128],
                    rhs=qp_sb[p][64:128, jq * 512:(jq + 1) * 512],
                    start=True, stop=True)
            pt_e = ptp.tile([128, KVG * 512], bf16, tag="pt", name="pt_e")
            nc.scalar.activation(pt_e[:], sc_e[:], EXP)
            pt_o = ptp.tile([128, KVG * 512], bf16, tag="pt", name="pt_o")
            nc.scalar.activation(pt_o[:], sc_o[:], EXP)
            for ki in range(KVG):
                kv = g * KVG + ki
                nc.tensor.matmul(
                    ctx_ps[0:HD + 1, 0:512],
                    lhsT=vn_tiles[h][kv][:, 0:HD + 1],
                    rhs=pt_e[:, ki * 512:(ki + 1) * 512],
                    start=(kv == 0), stop=(kv == NKV - 1))
                nc.tensor.matmul(
                    ctx_ps[0:HD + 1, 512:1024],
                    lhsT=vn_tiles[h][kv][:, 0:HD + 1],
                    rhs=pt_o[:, ki * 512:(ki + 1) * 512],
                    start=(kv == 0), stop=(kv == NKV - 1))

        def normalize(p, jq, ctx_ps):
            for hp in range(2):
                recip = smp.tile([1, 512], f32, tag="recip", name="recip")
                nc.vector.reciprocal(
                    recip[:], ctx_ps[HD:HD + 1, hp * 512:(hp + 1) * 512])
                rb = smp.tile([64, 512], f32, tag="rb", name="rb")
                nc.gpsimd.partition_broadcast(rb[:], recip[:])
                nc.vector.tensor_mul(
                    ctxT_sb[p][hp * 64:(hp + 1) * 64,
                               jq * 512:(jq + 1) * 512],
                    ctx_ps[0:64, hp * 512:(hp + 1) * 512], rb[:])

        # ---------- Phase A, with pair-0 jq-0 attention trickled in ----------
        ctx_ps0 = psc.tile([128, 1024], f32, tag="ctx", bufs=1, name="ctx_ps0")
        kv_quarter(0)
        dma_q_quarter(0)
        dma_wq()
        dma_q_quarter(1)
        for p in range(NP):
            q_unit(p, 0)
        kv_quarter(1)
        dma_q_quarter(2)
        b_group(0, 0, 0, ctx_ps0, 0)
        b_group(0, 0, 1, ctx_ps0, 0)
        for p in range(NP):
            q_unit(p, 1)
        kv_quarter(2)
        dma_q_quarter(3)
        dma_wo()
        b_group(0, 0, 2, ctx_ps0, 0)
        b_group(0, 0, 3, ctx_ps0, 0)
        for p in range(NP):
            q_unit(p, 2)
        kv_quarter(3)
        b_group(0, 0, 4, ctx_ps0, 0)
        b_group(0, 0, 5, ctx_ps0, 0)
        b_group(0, 0, 6, ctx_ps0, 0)
        b_group(0, 0, 7, ctx_ps0, 0)
        normalize(0, 0, ctx_ps0)

        # ---------- Main B loop with paced PE-filler units ----------
        segments = [[(1, 0), (2, 0), (3, 0)]] + [
            [(p, jq) for p in range(NP)] for jq in range(1, NJQ)]
        ost_tiles = {}
        for seg in segments:
            jq = seg[0][1]
            # filler units weighted by PE cost (q_unit = 16 matmuls -> 4
            # credits, c_unit = 4 matmuls -> 1 credit), paced per b_group.
            units = []
            if jq == 0:
                units += [(q_unit, (p, 3), 4) for p in range(NP)]
            else:
                for t in range((jq - 1) * 4, jq * 4):
                    ost = ostp.tile([128, E], bf16, tag="ost", name="ost")
                    units += [(c_unit, (t, e, ost), 1)
                              for e in range(E // 512)]
            total_credits = sum(u[2] for u in units)
            spent = 0
            uidx = 0
            ngroups = len(seg) * (NKV // KVG)
            gi = 0
            for p, jq in seg:
                h = p // 2
                ctx_ps = psc.tile([128, 1024], f32, tag="ctx", bufs=1,
                                  name="ctx_ps")
                for g in range(NKV // KVG):
                    b_group(p, jq, g, ctx_ps, h)
                    gi += 1
                    want = (gi * total_credits) // ngroups
                    while uidx < len(units) and spent < want:
                        fn, args, cost = units[uidx]
                        fn(*args)
                        spent += cost
                        uidx += 1
                normalize(p, jq, ctx_ps)
        for t in range((NJQ - 1) * 4, NJQ * 4):
            c_slab(t)
    nc.compile()
    return nc


def _get_nc():
    if "nc" not in _cache:
        _cache["nc"] = _build()
    return _cache["nc"]


def kernel(query, key, value, Wq, Wk, Wv, Wo, _trace=False):
    from concourse.bass_utils import run_bass_kernel_spmd

    def t_bf16(x):
        return np.ascontiguousarray(
            np.asarray(x, np.float32).astype(BF16).transpose(0, 2, 1))

    qT = t_bf16(query)
    kT = t_bf16(key)
    vT = t_bf16(value)
    Wq = np.asarray(Wq, np.float32).astype(BF16)
    Wk = np.asarray(Wk, np.float32).astype(BF16)
    Wv = np.asarray(Wv, np.float32).astype(BF16)
    Wo = np.asarray(Wo, np.float32).astype(BF16)

    in_maps = []
    for c in range(NCORES):
        b, g = divmod(c, NTPG)
        in_maps.append({
            "qT": np.ascontiguousarray(qT[b]),
            "kT": np.ascontiguousarray(kT[b]),
            "vT": np.ascontiguousarray(vT[b]),
            "wq": np.ascontiguousarray(Wq[:, g * QD:(g + 1) * QD]),
            "wk": np.ascontiguousarray(Wk[:, g * KD:(g + 1) * KD]),
            "wv": np.ascontiguousarray(Wv[:, g * KD:(g + 1) * KD]),
            "wo": np.ascontiguousarray(Wo[g * QD:(g + 1) * QD, :]),
        })

    nc = _get_nc()
    res = run_bass_kernel_spmd(nc, in_maps, list(range(NCORES)), trace=_trace)
    out = np.zeros((B, S, E), np.float32)
    for c in range(NCORES):
        out[c // NTPG] += res.results[c]["out"].astype(np.float32)
    if _trace:
        _cache["last_exec_time_ns"] = res.exec_time_ns
        _cache["last_results"] = res
    return out


# revision 21
# speedup vs baseline: 1.2893x; 1.0676x over previous
"""GQA kernel for 8 trn2 NeuronCores — v6 (DP x TP).

Sharding: 2-way data-parallel over batch x 4-way tensor-parallel over KV
groups. Core c owns batch c//4 and KV group g=c%4: kv heads {2g, 2g+1},
q heads {8g..8g+7} (cols 512g:512g+512 of Wq, cols 128g:128g+128 of
Wk/Wv, rows 512g:512g+512 of Wo). Each core computes a partial output
[S, E]; host sums 4 partials per batch.

DMA instruction count is minimized (HWDGE costs ~630ns serialized per
DMA): inputs land as [128, 4, 512] 4-chunk groups, weights in 1-4
transfers each, outputs batched per 128-row slab.

Device algorithm (single batch per core):
  A. per seq-quarter t: DMA kT/vT/qT groups; K.T projection (M=128,
     both kv heads packed), V projected DIRECTLY in natural layout
     (lhsT = vT chunk, rhs = Wv) -> per-chunk per-head V_aug [128, 65]
     tiles (ones col appended) with no DMA transposes; Q pairs
     projected as the needed qT groups arrive. K.T per head duplicated
     into the other partition half per quarter (SBUF->SBUF DMA) so
     score matmuls can row-tile both heads of a pair. Pair-0 jq-0
     attention groups are trickled between quarters to fill PE.
  B. remaining (pair, jq); per (pair, jq) 8 groups of 2 kv chunks:
     row-tiled score pair -> exp (ACT, PSUM->SBUF bf16) -> ctx.T
     accumulation (row 64 = softmax denominator). Between groups,
     PE-filler units (Q proj t=3, out-proj chunks of jq-1) keep PE busy
     while ACT drains exp (exp runs only on the Activation engine).
  C. out slab t: out[t*128:+128, :] = sum_p ctxT_p.T @ Wo_p, 4 PSUM
     accs + copies into one [128, 2048] tile, single DMA.

All matmuls bf16 inputs / fp32 PSUM. PSUM banks: acc(2x1) + sc(2x2) +
ctx(2) = 8.
"""

import numpy as np
import ml_dtypes

B = 2
S = 2048
E = 2048
HD = 64           # head dim
HPC = 8           # q heads per core
NP = 4            # head pairs per core
NKVH = 2          # kv heads per core
QD = HPC * HD     # 512 per-core q dims
KD = NKVH * HD    # 128 per-core kv dims
NCORES = 8
NTPG = 4          # tensor-parallel group size (cores per batch)
EC = E // 128     # 16 contraction chunks
EG = 4            # e-chunks per input DMA group
NEG = EC // EG    # 4 groups
NJQ = S // 512    # 4 q-chunks of 512
NKV = S // 128    # 16 kv chunks of 128
KVG = 2           # kv chunks per exp group
BF16 = ml_dtypes.bfloat16

_cache = {}


def _build():
    from contextlib import ExitStack
    from concourse import bacc, tile
    import concourse.mybir as mybir

    bf16 = mybir.dt.bfloat16
    f32 = mybir.dt.float32
    EXP = mybir.ActivationFunctionType.Exp

    nc = bacc.Bacc(
        "TRN2", target_bir_lowering=False, debug=False, num_devices=NCORES)
    qT_d = nc.declare_dram_parameter("qT", [E, S], bf16, isOutput=False)
    kT_d = nc.declare_dram_parameter("kT", [E, S], bf16, isOutput=False)
    vT_d = nc.declare_dram_parameter("vT", [E, S], bf16, isOutput=False)
    wq_d = nc.declare_dram_parameter("wq", [E, QD], bf16, isOutput=False)
    wk_d = nc.declare_dram_parameter("wk", [E, KD], bf16, isOutput=False)
    wv_d = nc.declare_dram_parameter("wv", [E, KD], bf16, isOutput=False)
    wo_d = nc.declare_dram_parameter("wo", [QD, E], bf16, isOutput=False)
    out_d = nc.declare_dram_parameter("out", [S, E], bf16, isOutput=True)

    with ExitStack() as ctx:
        tc = ctx.enter_context(tile.TileContext(nc))
        # ---- pools ----
        wpool = ctx.enter_context(tc.tile_pool(name="w", bufs=1))
        qin = ctx.enter_context(tc.tile_pool(name="qin", bufs=16))
        kvin = ctx.enter_context(tc.tile_pool(name="kvin", bufs=4))
        qts = ctx.enter_context(tc.tile_pool(name="qts", bufs=1))
        vnp = ctx.enter_context(tc.tile_pool(name="vnp", bufs=32))
        ptp = ctx.enter_context(tc.tile_pool(name="ptp", bufs=4))
        ostp = ctx.enter_context(tc.tile_pool(name="ostp", bufs=2))
        smp = ctx.enter_context(tc.tile_pool(name="smp", bufs=2))
        psa = ctx.enter_context(tc.tile_pool(name="psa", bufs=2, space="PSUM"))
        psc = ctx.enter_context(tc.tile_pool(name="psc", bufs=2, space="PSUM"))

        # ---- weights: few big DMAs on the ACT queue (idle in phase A) ----
        wk_sb = wpool.tile([128, EC, KD], bf16)
        nc.scalar.dma_start(wk_sb[:], wk_d.rearrange("(c p) m -> p c m", p=128))
        wv_sb = wpool.tile([128, EC, KD], bf16)
        nc.scalar.dma_start(wv_sb[:], wv_d.rearrange("(c p) m -> p c m", p=128))
        wq_sb = wpool.tile([128, EC, QD], bf16)
        wq_r = wq_d.rearrange("(c p) m -> p c m", p=128)
        wo_sb = wpool.tile([128, NP, E], bf16)
        wo_r = wo_d.rearrange("(c p) e -> p c e", p=128)

        def dma_wq():
            for g in range(NEG):
                nc.scalar.dma_start(
                    wq_sb[:, g * EG:(g + 1) * EG, :], wq_r[:, g * EG:(g + 1) * EG, :])

        def dma_wo():
            for kc in range(0, NP, 2):
                nc.scalar.dma_start(
                    wo_sb[:, kc:kc + 2, :], wo_r[:, kc:kc + 2, :])

        # ---- persistent SBUF tiles ----
        # Q pair tiles: heads 2p at partitions 0:64, 2p+1 at 64:128
        qp_sb = [qts.tile([128, S], bf16, tag=f"qp{p}", name=f"qp{p}")
                 for p in range(NP)]
        # K.T per kv head, duplicated across both partition halves
        kt2_sb = [qts.tile([128, S], bf16, tag=f"kt2{h}", name=f"kt2{h}")
                  for h in range(NKVH)]
        # ctx.T pair tiles
        ctxT_sb = [qts.tile([128, S], bf16, tag=f"ctxT{p}", name=f"ctxT{p}")
                   for p in range(NP)]
        # V natural + ones col, per kv head per kv chunk
        vn_tiles = [[None] * NKV for _ in range(NKVH)]
        # qT group tiles [128, EG, 512] per (g, t), resident for Q proj
        qt_tiles = [[None] * NJQ for _ in range(NEG)]

        def dma_q_quarter(t):
            # qT on the ACT queue: fully resident (16 bufs), no ring waits.
            for g in range(NEG):
                qt = qin.tile([128, EG, 512], bf16, tag="qt", name="qt")
                nc.scalar.dma_start(
                    qt[:],
                    qT_d[g * EG * 128:(g + 1) * EG * 128,
                         t * 512:(t + 1) * 512].rearrange(
                             "(g p) s -> p g s", p=128))
                qt_tiles[g][t] = qt

        def q_unit(p, t):
            """Project Q pair p at seq quarter t (atomic: 16 matmuls)."""
            acc = psa.tile([128, 512], f32, tag="acc", name="qacc")
            for e in range(EC):
                nc.tensor.matmul(
                    acc[:], lhsT=wq_sb[:, e, p * 128:(p + 1) * 128],
                    rhs=qt_tiles[e // EG][t][:, e % EG, :],
                    start=(e == 0), stop=(e == EC - 1))
            nc.vector.tensor_scalar_mul(
                qp_sb[p][:, t * 512:(t + 1) * 512], acc[:], 0.125)

        def kv_quarter(t):
            """DMA + project K (transposed) and V (natural) at quarter t."""
            kins, vins = [], []
            for g in range(NEG):
                ki = kvin.tile([128, EG, 512], bf16, tag="kin", name="kin")
                nc.sync.dma_start(
                    ki[:],
                    kT_d[g * EG * 128:(g + 1) * EG * 128,
                         t * 512:(t + 1) * 512].rearrange(
                             "(g p) s -> p g s", p=128))
                kins.append(ki)
                vi = kvin.tile([128, EG, 512], bf16, tag="vin", name="vin")
                nc.sync.dma_start(
                    vi[:],
                    vT_d[g * EG * 128:(g + 1) * EG * 128,
                         t * 512:(t + 1) * 512].rearrange(
                             "(g p) s -> p g s", p=128))
                vins.append(vi)
            kacc = psa.tile([128, 512], f32, tag="acc", name="kacc")
            for e in range(EC):
                nc.tensor.matmul(kacc[:], lhsT=wk_sb[:, e, :],
                                 rhs=kins[e // EG][:, e % EG, :],
                                 start=(e == 0), stop=(e == EC - 1))
            # head 0 -> kt2[0] partitions 0:64, head 1 -> kt2[1] 64:128
            nc.vector.tensor_copy(
                kt2_sb[0][0:64, t * 512:(t + 1) * 512], kacc[0:64, :])
            nc.vector.tensor_copy(
                kt2_sb[1][64:128, t * 512:(t + 1) * 512], kacc[64:128, :])
            # V natural: per kv chunk, out[seq 128, kvdim 128] then split
            # per head into V_aug tiles (ones column appended).
            for c in range(t * 4, t * 4 + 4):
                vacc = psa.tile([128, 512], f32, tag="acc", name="vacc")
                for e in range(EC):
                    nc.tensor.matmul(
                        vacc[:, 0:KD],
                        lhsT=vins[e // EG][:, e % EG,
                                           (c % 4) * 128:(c % 4) * 128 + 128],
                        rhs=wv_sb[:, e, :],
                        start=(e == 0), stop=(e == EC - 1))
                for h in range(NKVH):
                    vn = vnp.tile([128, HD + 1], bf16, tag="vn", name="vn")
                    nc.vector.tensor_copy(
                        vn[:, 0:HD], vacc[:, h * HD:(h + 1) * HD])
                    nc.vector.memset(vn[:, HD:HD + 1], 1.0)
                    vn_tiles[h][c] = vn
            # per-quarter K.T duplication so pair-0 attention can start
            # before the full K row is assembled
            nc.sync.dma_start(kt2_sb[0][64:128, t * 512:(t + 1) * 512],
                              kt2_sb[0][0:64, t * 512:(t + 1) * 512])
            nc.sync.dma_start(kt2_sb[1][0:64, t * 512:(t + 1) * 512],
                              kt2_sb[1][64:128, t * 512:(t + 1) * 512])

        def c_slab(t):
            """out[t*128:+128, :]: 4 accs + 4 copies into one tile, 1 DMA."""
            ost = ostp.tile([128, E], bf16, tag="ost", name="ost")
            for e in range(E // 512):
                ops = psa.tile([128, 512], f32, tag="acc", name="cacc")
                for kc in range(NP):
                    nc.tensor.matmul(
                        ops[:], lhsT=ctxT_sb[kc][:, t * 128:(t + 1) * 128],
                        rhs=wo_sb[:, kc, e * 512:(e + 1) * 512],
                        start=(kc == 0), stop=(kc == NP - 1))
                nc.vector.tensor_copy(ost[:, e * 512:(e + 1) * 512], ops[:])
            nc.sync.dma_start(out_d[t * 128:(t + 1) * 128, :], ost[:])

        def c_unit(t, e, ost):
            ops = psa.tile([128, 512], f32, tag="acc", name="cacc")
            for kc in range(NP):
                nc.tensor.matmul(
                    ops[:], lhsT=ctxT_sb[kc][:, t * 128:(t + 1) * 128],
                    rhs=wo_sb[:, kc, e * 512:(e + 1) * 512],
                    start=(kc == 0), stop=(kc == NP - 1))
            nc.vector.tensor_copy(ost[:, e * 512:(e + 1) * 512], ops[:])
            if e == E // 512 - 1:
                nc.sync.dma_start(out_d[t * 128:(t + 1) * 128, :], ost[:])

        # ---------- Phase B building blocks ----------
        # ctx is software-pipelined one group behind scores/exp: when PE
        # reaches ctx(g-1), its exp finished a full period ago, so the
        # 4-deep PE wait queue never fills and the sequencer doesn't stall.
        def b_scores(p, jq, g, h):
            sc_e = psc.tile([128, KVG * 512], f32, tag="sc", name="sc_e")
            sc_o = psc.tile([128, KVG * 512], f32, tag="sc", name="sc_o")
            for ki in range(KVG):
                kv = g * KVG + ki
                nc.tensor.matmul(
                    sc_e[:, ki * 512:(ki + 1) * 512],
                    lhsT=kt2_sb[h][0:64, kv * 128:(kv + 1) * 128],
                    rhs=qp_sb[p][0:64, jq * 512:(jq + 1) * 512],
                    start=True, stop=True)
                nc.tensor.matmul(
                    sc_o[:, ki * 512:(ki + 1) * 512],
                    lhsT=kt2_sb[h][64:128, kv * 128:(kv + 1) * 128],
                    rhs=qp_sb[p][64:128, jq * 512:(jq + 1) * 512],
                    start=True, stop=True)
            pt_e = ptp.tile([128, KVG * 512], bf16, tag="pt", name="pt_e")
            nc.scalar.activation(pt_e[:], sc_e[:], EXP)
            pt_o = ptp.tile([128, KVG * 512], bf16, tag="pt", name="pt_o")
            nc.scalar.activation(pt_o[:], sc_o[:], EXP)
            return pt_e, pt_o

        def b_ctx(g, pts, ctx_ps, h):
            pt_e, pt_o = pts
            for ki in range(KVG):
                kv = g * KVG + ki
                nc.tensor.matmul(
                    ctx_ps[0:HD + 1, 0:512],
                    lhsT=vn_tiles[h][kv][:, 0:HD + 1],
                    rhs=pt_e[:, ki * 512:(ki + 1) * 512],
                    start=(kv == 0), stop=(kv == NKV - 1))
                nc.tensor.matmul(
                    ctx_ps[0:HD + 1, 512:1024],
                    lhsT=vn_tiles[h][kv][:, 0:HD + 1],
                    rhs=pt_o[:, ki * 512:(ki + 1) * 512],
                    start=(kv == 0), stop=(kv == NKV - 1))

        def normalize(p, jq, ctx_ps):
            for hp in range(2):
                recip = smp.tile([1, 512], f32, tag="recip", name="recip")
                nc.vector.reciprocal(
                    recip[:], ctx_ps[HD:HD + 1, hp * 512:(hp + 1) * 512])
                rb = smp.tile([64, 512], f32, tag="rb", name="rb")
                nc.gpsimd.partition_broadcast(rb[:], recip[:])
                nc.vector.tensor_mul(
                    ctxT_sb[p][hp * 64:(hp + 1) * 64,
                               jq * 512:(jq + 1) * 512],
                    ctx_ps[0:64, hp * 512:(hp + 1) * 512], rb[:])

        # ---------- Phase A, with pair-0 jq-0 attention trickled in ----------
        ctx_ps0 = psc.tile([128, 1024], f32, tag="ctx", bufs=1, name="ctx_ps0")
        pend = [None]

        def trick(g):
            pts = b_scores(0, 0, g, 0)
            if pend[0] is not None:
                b_ctx(g - 1, pend[0], ctx_ps0, 0)
            pend[0] = pts

        kv_quarter(0)
        dma_q_quarter(0)
        dma_wq()
        dma_q_quarter(1)
        for p in range(NP):
            q_unit(p, 0)
        kv_quarter(1)
        dma_q_quarter(2)
        trick(0)
        trick(1)
        kv_quarter(2)
        dma_q_quarter(3)
        dma_wo()
        trick(2)
        trick(3)
        kv_quarter(3)
        trick(4)
        trick(5)
        trick(6)
        trick(7)
        b_ctx(7, pend[0], ctx_ps0, 0)
        normalize(0, 0, ctx_ps0)

        # ---------- Main B loop with paced PE-filler units ----------
        segments = [[(1, 0), (2, 0), (3, 0)]] + [
            [(p, jq) for p in range(NP)] for jq in range(1, NJQ)]
        ost_tiles = {}
        for seg in segments:
            jq = seg[0][1]
            # filler units weighted by PE cost (q_unit = 16 matmuls -> 4
            # credits, c_unit = 4 matmuls -> 1 credit), paced per b_group.
            units = []
            if jq == 0:
                units += [(q_unit, (p, 3), 4) for p in range(NP)]
            else:
                for t in range((jq - 1) * 4, jq * 4):
                    ost = ostp.tile([128, E], bf16, tag="ost", name="ost")
                    units += [(c_unit, (t, e, ost), 1)
                              for e in range(E // 512)]
            total_credits = sum(u[2] for u in units)
            spent = 0
            uidx = 0
            # 9 pacing points per (pair, jq): 8 groups + one before the
            # trailing ctx(7), which waits on the freshest exp.
            npts = len(seg) * (NKV // KVG + 1)
            gi = 0

            def fill():
                nonlocal spent, uidx
                want = (gi * total_credits) // npts
                while uidx < len(units) and spent < want:
                    fn, args, cost = units[uidx]
                    fn(*args)
                    spent += cost
                    uidx += 1

            for p, jq in seg:
                h = p // 2
                ctx_ps = psc.tile([128, 1024], f32, tag="ctx", bufs=1,
                                  name="ctx_ps")
                pts_prev = None
                for g in range(NKV // KVG):
                    pts = b_scores(p, jq, g, h)
                    if pts_prev is not None:
                        b_ctx(g - 1, pts_prev, ctx_ps, h)
                    pts_prev = pts
                    gi += 1
                    fill()
                gi += 1
                fill()
                b_ctx(NKV // KVG - 1, pts_prev, ctx_ps, h)
                normalize(p, jq, ctx_ps)
        for t in range((NJQ - 1) * 4, NJQ * 4):
            c_slab(t)
    nc.compile()
    return nc


def _get_nc():
    if "nc" not in _cache:
        _cache["nc"] = _build()
    return _cache["nc"]


def kernel(query, key, value, Wq, Wk, Wv, Wo, _trace=False):
    from concourse.bass_utils import run_bass_kernel_spmd

    def t_bf16(x):
        return np.ascontiguousarray(
            np.asarray(x, np.float32).astype(BF16).transpose(0, 2, 1))

    qT = t_bf16(query)
    kT = t_bf16(key)
    vT = t_bf16(value)
    Wq = np.asarray(Wq, np.float32).astype(BF16)
    Wk = np.asarray(Wk, np.float32).astype(BF16)
    Wv = np.asarray(Wv, np.float32).astype(BF16)
    Wo = np.asarray(Wo, np.float32).astype(BF16)

    in_maps = []
    for c in range(NCORES):
        b, g = divmod(c, NTPG)
        in_maps.append({
            "qT": np.ascontiguousarray(qT[b]),
            "kT": np.ascontiguousarray(kT[b]),
            "vT": np.ascontiguousarray(vT[b]),
            "wq": np.ascontiguousarray(Wq[:, g * QD:(g + 1) * QD]),
            "wk": np.ascontiguousarray(Wk[:, g * KD:(g + 1) * KD]),
            "wv": np.ascontiguousarray(Wv[:, g * KD:(g + 1) * KD]),
            "wo": np.ascontiguousarray(Wo[g * QD:(g + 1) * QD, :]),
        })

    nc = _get_nc()
    res = run_bass_kernel_spmd(nc, in_maps, list(range(NCORES)), trace=_trace)
    out = np.zeros((B, S, E), np.float32)
    for c in range(NCORES):
        out[c // NTPG] += res.results[c]["out"].astype(np.float32)
    if _trace:
        _cache["last_exec_time_ns"] = res.exec_time_ns
        _cache["last_results"] = res
    return out
